# revision 1
# baseline (speedup 1.0000x reference)
"""Multi-head attention TRN2 Bass kernel.

Problem: B=4, S=2048, D=1024, H=16 heads (DK=64), fp32, random 0/1
attention mask broadcast over heads.

Sharding: 8 cores = (batch b, query-half) pairs. Core c handles batch
c//2, query rows [ (c%2)*1024, (c%2+1)*1024 ).  K/V projections for the
batch are computed redundantly on the 2 cores sharing a batch; no
collectives are needed and each core writes a disjoint output slice.

Layout strategy (per core):
  - Host pre-transposes q, k, v (and the mask) so the feature dim lands
    on SBUF partitions; projections then run without any on-chip
    transposes.
  - qhT [D, SQ], khT [D, S] are produced transposed (out-feature on
    partitions) and spilled to DRAM; vh is produced in natural layout
    [S, D] extended with a ones column per head ([vh_h | 1], width 65).
  - scoresT_chunk[k, q] = khT_chunk.T @ qhT  (k on partitions) via
    matmul(lhsT=khT[64, 128chunk], rhs=qhT[64, 512]).
  - exp on ACT (no max subtraction: scores/8 are O(1), mask applied
    multiplicatively post-exp), mask-multiply on DVE.
  - AV: matmul(lhsT=[vh_h | 1][128, 65], rhs=mexpT[128, 512]) accumulated
    over 16 k-chunks -> PSUM [65, 1024]: rows 0..63 = unnormalized out^T,
    row 64 = softmax denominators.
  - normalize: DMA-broadcast denominators to 64 partitions (via a DRAM
    round-trip: SBUF-source DMAs cannot have a zero partition step), DVE
    reciprocal + multiply; result IS the transposed lhsT for the output
    projection.

All matmuls run as float32r (TF32-like rounding, 1 row/cycle vs 4 for
fp32; bit-identical storage).  Measured on 8 axon trn2 cores:
relative error 1.6e-04 vs the fp32 jax reference, ~1.07 ms/exec
(marginal cost of an extra pipelined NEFF execution, all I/O in DRAM).
"""

import os
import sys

if "/opt/trn_rl_repo" not in sys.path:
    sys.path.insert(0, "/opt/trn_rl_repo")
os.environ.setdefault("MYCRO_LOCAL_CACHE", "1")

import numpy as np
import ml_dtypes

import concourse.bass as bass
import concourse.bacc as bacc
import concourse.mybir as mybir
import concourse.tile as tile
from concourse.bass import ts

B, S, D, H, DK = 4, 2048, 1024, 16, 64
SQ = S // 2          # q rows per core
P = 128
NCHUNK = S // P      # 16 k-chunks
NJ = D // P          # 8 feature chunks
NQT = SQ // P        # 8 q tiles
N_CORES = 8

F32 = mybir.dt.float32
BF16 = mybir.dt.bfloat16
AF = mybir.ActivationFunctionType

# matmul compute dtype: float32r streams 1 row/cycle (vs 4 for float32)
# at N>=256.  Bits are identical to fp32; precision measured on HW.
# MD is the dtype of every tensor that feeds a matmul operand.  float32r
# is required to be plumbed through producer output dtypes (walrus BIR
# verifier: "consumed by FP32r matmult but is not rounded to FP32r").
FAST = os.environ.get("MHA_MM_DT", "f32r") == "f32r"
MD = mybir.dt.float32r if FAST else mybir.dt.float32


def _mm(ap):
    return ap


def build_program(n_iters=1):
    nc = bacc.Bacc(
        "TRN2",
        target_bir_lowering=False,
        debug=False,
        enable_asserts=False,
    )

    # ---- DRAM I/O (per-core slices; host pre-transposed) ----
    qT_d = nc.dram_tensor("qT", [D, SQ], MD, kind="ExternalInput").ap()
    kT_d = nc.dram_tensor("kT", [D, S], MD, kind="ExternalInput").ap()
    vT_d = nc.dram_tensor("vT", [D, S], MD, kind="ExternalInput").ap()
    mT_d = nc.dram_tensor("maskT", [S, SQ], BF16, kind="ExternalInput").ap()
    wq_d = nc.dram_tensor("wq", [D, D], MD, kind="ExternalInput").ap()
    wk_d = nc.dram_tensor("wk", [D, D], MD, kind="ExternalInput").ap()
    wv_d = nc.dram_tensor("wv", [D, D], MD, kind="ExternalInput").ap()
    wo_d = nc.dram_tensor("wo", [D, D], MD, kind="ExternalInput").ap()
    bq_d = nc.dram_tensor("bq", [D], F32, kind="ExternalInput").ap()
    bk_d = nc.dram_tensor("bk", [D], F32, kind="ExternalInput").ap()
    bv_d = nc.dram_tensor("bv", [D], MD, kind="ExternalInput").ap()
    bo_d = nc.dram_tensor("bo", [D], MD, kind="ExternalInput").ap()
    ones_d = nc.dram_tensor("ones_row", [1, P], MD, kind="ExternalInput").ap()
    ones16_d = nc.dram_tensor("ones16", [P, H], MD, kind="ExternalInput").ap()
    out_d = nc.dram_tensor("out", [SQ, D], F32, kind="ExternalOutput").ap()

    with tile.TileContext(nc) as tc:
        for _ in range(n_iters):
            _build(nc, tc, qT_d, kT_d, vT_d, mT_d,
                   wq_d, wk_d, wv_d, wo_d, bq_d, bk_d, bv_d, bo_d, out_d,
                   ones_d, ones16_d)

    nc.compile()
    return nc


def _proj_T(nc, tc, ctx, x_d, w_d, bias_col, out_sink, scale, n_src, tagp=""):
    """Transposed projection: out[j, r] = sum_d w[d, j] * xT[d, r] + b[j].

    x_d: DRAM [D, n_src] (input, pre-transposed).  w_d: DRAM [D, D].
    bias_col: SBUF [P, NJ] per-feature bias columns (already scaled).
    out_sink(j, half, stage_ap): consume the [P, SQ] result slab for
    output features [j*128, (j+1)*128) and source rows
    [half*SQ, half*SQ+SQ).
    """
    nhalf = n_src // SQ
    win = ctx.enter_context(tc.tile_pool(name=f"win{tagp}", bufs=1))
    xT = [win.tile([P, SQ], MD, tag=f"xT{tagp}{d}", name=f"xT{tagp}{d}")
          for d in range(NJ)]
    wti = [win.tile([P, D], MD, tag=f"wti{tagp}{d}", name=f"wti{tagp}{d}")
           for d in range(NJ)]
    stage = ctx.enter_context(tc.tile_pool(name=f"stage{tagp}", bufs=3))
    psum = ctx.enter_context(tc.tile_pool(name=f"ps{tagp}", bufs=2, space="PSUM"))
    for d in range(NJ):
        nc.sync.dma_start(wti[d][:], w_d[ts(d, P), :])
    for half in range(nhalf):
        for d in range(NJ):
            nc.sync.dma_start(xT[d][:], x_d[ts(d, P), ts(half, SQ)])
        for j in range(NJ):
            pq = psum.tile([P, SQ], F32, tag="pq")
            for d in range(NJ):
                for h2 in range(SQ // 512):
                    nc.tensor.matmul(
                        pq[:, ts(h2, 512)],
                        _mm(wti[d][:, ts(j, P)]),
                        _mm(xT[d][:, ts(h2, 512)]),
                        start=(d == 0), stop=(d == NJ - 1),
                    )
            st = stage.tile([P, SQ], MD, tag="stp")
            nc.scalar.activation(st[:], pq[:], AF.Identity,
                                 bias=bias_col[:, ts(j, 1)], scale=scale)
            out_sink(j, half, st)


def _build(nc, tc, qT_d, kT_d, vT_d, mT_d,
           wq_d, wk_d, wv_d, wo_d, bq_d, bk_d, bv_d, bo_d, out_d,
           ones_d, ones16_d):
    from contextlib import ExitStack

    with ExitStack() as top:
        dram = top.enter_context(tc.tile_pool(name="dram", bufs=1, space="DRAM"))
        qhT_dram = dram.tile([NJ, P, SQ], MD)   # q-head projections, transposed
        khT_dram = dram.tile([NJ, P, S], MD)    # k-head projections, transposed
        ctT_dram = dram.tile([NJ, P, SQ], MD)   # normalized attention out^T
        sums_dram = dram.tile([H, SQ], F32)      # per-head softmax denominators

        consts = top.enter_context(tc.tile_pool(name="consts", bufs=1))
        ones_row = consts.tile([1, P], MD, tag="ones_row")
        nc.sync.dma_start(ones_row[:], ones_d)
        bv_row = consts.tile([1, D], MD, tag="bv_row")
        nc.sync.dma_start(bv_row[:], bv_d.rearrange("(o n) -> o n", o=1))
        bo_row = consts.tile([1, D], MD, tag="bo_row")
        nc.sync.dma_start(bo_row[:], bo_d.rearrange("(o n) -> o n", o=1))
        # per-chunk bias columns [128, 1]
        bq_c = consts.tile([P, NJ], F32, tag="bq_c")
        nc.sync.dma_start(bq_c[:], bq_d.rearrange("(j p) -> p j", p=P))
        nc.vector.tensor_scalar_mul(bq_c[:], bq_c[:], 0.125)  # fold 1/sqrt(DK)
        bk_c = consts.tile([P, NJ], F32, tag="bk_c")
        nc.sync.dma_start(bk_c[:], bk_d.rearrange("(j p) -> p j", p=P))

        # W_o preloaded here so phase G never stalls on weight DMA.
        wo_pool = top.enter_context(tc.tile_pool(name="wo", bufs=1))
        wo_t = [wo_pool.tile([P, D], MD, tag=f"wo{d}", name=f"wo{d}")
                for d in range(NJ)]
        for d in range(NJ):
            nc.sync.dma_start(wo_t[d][:], wo_d[ts(d, P), :])

        # ---------- Phases B+C: q and k projections (distinct pools so
        # C's DMA loads overlap B's matmuls) ----------
        with ExitStack() as ctx:
            def q_sink(j, half, st):
                nc.sync.dma_start(qhT_dram[j], st[:])
            _proj_T(nc, tc, ctx, qT_d, wq_d, bq_c, q_sink, 0.125, SQ, tagp="q")

            def k_sink(j, half, st):
                nc.sync.dma_start(khT_dram[j][:, ts(half, SQ)], st[:])
            _proj_T(nc, tc, ctx, kT_d, wk_d, bk_c, k_sink, 1.0, S, tagp="k")

        # persistent attention operands (allocated after B/C pools closed)
        kv_pool = top.enter_context(tc.tile_pool(name="kv", bufs=1))
        vh_ext = [kv_pool.tile([P, H * (DK + 1)], MD, tag=f"vhe{c}",
                               name=f"vhe{c}") for c in range(NCHUNK)]
        maskT = [kv_pool.tile([P, SQ], BF16, tag=f"mT{c}", name=f"mT{c}")
                 for c in range(NCHUNK)]

        # ---------- Phase D: v projection -> vh_ext (resident) ----------
        with ExitStack() as ctx:
            win = ctx.enter_context(tc.tile_pool(name="win", bufs=1))
            vT = [win.tile([P, SQ], MD, tag=f"vT{d}", name=f"vT{d}")
                  for d in range(NJ)]
            wti = [win.tile([P, D], MD, tag=f"wti{d}", name=f"wti{d}")
                   for d in range(NJ)]
            psum = ctx.enter_context(tc.tile_pool(name="psD", bufs=2, space="PSUM"))
            for d in range(NJ):
                nc.sync.dma_start(wti[d][:], wv_d[ts(d, P), :])
            for vhalf in range(2):
                for d in range(NJ):
                    nc.sync.dma_start(vT[d][:], vT_d[ts(d, P), ts(vhalf, SQ)])
                for cl in range(NCHUNK // 2):
                    c = vhalf * (NCHUNK // 2) + cl
                    nc.sync.dma_start(
                        vh_ext[c].rearrange("p (h w) -> p h w", w=DK + 1)[:, :, DK:DK + 1],
                        ones16_d.rearrange("p (h o) -> p h o", o=1),
                    )
                    pv = psum.tile([P, D], F32, tag="pv")
                    for d in range(NJ):
                        for h2 in range(D // 512):
                            nc.tensor.matmul(
                                pv[:, ts(h2, 512)],
                                _mm(vT[d][:, ts(cl, P)]),
                                _mm(wti[d][:, ts(h2, 512)]),
                                start=(d == 0), stop=False,
                            )
                    for h2 in range(D // 512):
                        nc.tensor.matmul(
                            pv[:, ts(h2, 512)],
                            _mm(ones_row[:]),
                            _mm(bv_row[:, ts(h2, 512)]),
                            start=False, stop=True,
                        )
                    for h in range(H):
                        nc.vector.tensor_copy(
                            vh_ext[c][:, h * (DK + 1): h * (DK + 1) + DK],
                            pv[:, ts(h, DK)],
                        )

        # ---------- Phase E: mask load (host pre-transposed bf16) ----------
        for c in range(NCHUNK):
            nc.sync.dma_start(maskT[c][:], mT_d[ts(c, P), :])

        # ---------- Phase F: attention ----------
        with ExitStack() as ctx:
            qp = ctx.enter_context(tc.tile_pool(name="qp", bufs=2))
            kp = ctx.enter_context(tc.tile_pool(name="kp", bufs=2))
            ep = ctx.enter_context(tc.tile_pool(name="ep", bufs=2))
            mp = ctx.enter_context(tc.tile_pool(name="mp", bufs=3))
            rp = ctx.enter_context(tc.tile_pool(name="rp", bufs=2))
            ps_s = ctx.enter_context(tc.tile_pool(name="ps_s", bufs=2, space="PSUM"))
            ps_a = ctx.enter_context(tc.tile_pool(name="ps_a", bufs=2, space="PSUM"))
            for hp in range(H // 2):
                qhT_pair = qp.tile([P, SQ], MD, tag="qhT_pair")
                nc.sync.dma_start(qhT_pair[:], qhT_dram[hp])
                khT_pair = kp.tile([P, S], MD, tag="khT_pair")
                nc.sync.dma_start(khT_pair[:], khT_dram[hp])
                for hh in range(2):
                    h = 2 * hp + hh
                    pa = ps_a.tile([DK + 1, SQ], F32, tag="pa")
                    for c in range(NCHUNK):
                        pscr = ps_s.tile([P, SQ], F32, tag="pscr")
                        for h2 in range(SQ // 512):
                            nc.tensor.matmul(
                                pscr[:, ts(h2, 512)],
                                _mm(khT_pair[ts(hh, DK), ts(c, P)]),
                                _mm(qhT_pair[ts(hh, DK), ts(h2, 512)]),
                                start=True, stop=True,
                            )
                        et = ep.tile([P, SQ], F32, tag="et")
                        nc.scalar.activation(et[:], pscr[:], AF.Exp)
                        mt = mp.tile([P, SQ], MD, tag="mt")
                        nc.vector.tensor_mul(mt[:], et[:], maskT[c][:])
                        for h2 in range(SQ // 512):
                            nc.tensor.matmul(
                                pa[:, ts(h2, 512)],
                                _mm(vh_ext[c][:, h * (DK + 1): (h + 1) * (DK + 1)]),
                                _mm(mt[:, ts(h2, 512)]),
                                start=(c == 0), stop=(c == NCHUNK - 1),
                            )
                    # normalize: rows 0..63 /= row 64
                    sums = rp.tile([1, SQ], F32, tag="sums")
                    nc.vector.tensor_copy(sums[:], pa[DK:DK + 1, :])
                    nc.sync.dma_start(
                        sums_dram[h].rearrange("(o n) -> o n", o=1), sums[:])
                    rb = rp.tile([DK, SQ], F32, tag="rb")
                    nc.sync.dma_start(
                        rb[:], sums_dram[h].rearrange("(o n) -> o n", o=1)
                        .to_broadcast((DK, SQ)))
                    nc.vector.reciprocal(rb[:], rb[:])
                    ot = rp.tile([DK, SQ], MD, tag="ot")
                    nc.vector.tensor_mul(ot[:], pa[0:DK, :], rb[:])
                    nc.sync.dma_start(ctT_dram[hp, ts(hh, DK), :], ot[:])

        # ---------- Phase G: output projection ----------
        with ExitStack() as ctx:
            win = ctx.enter_context(tc.tile_pool(name="winG", bufs=1))
            ctT = [win.tile([P, SQ], MD, tag=f"ctT{d}", name=f"ctT{d}")
                   for d in range(NJ)]
            wti = wo_t
            stage = ctx.enter_context(tc.tile_pool(name="stageG", bufs=2))
            psum = ctx.enter_context(tc.tile_pool(name="psG", bufs=2, space="PSUM"))
            for d in range(NJ):
                nc.sync.dma_start(ctT[d][:], ctT_dram[d])
            for qt in range(NQT):
                po = psum.tile([P, D], F32, tag="po")
                for d in range(NJ):
                    for h2 in range(D // 512):
                        nc.tensor.matmul(
                            po[:, ts(h2, 512)],
                            _mm(ctT[d][:, ts(qt, P)]),
                            _mm(wti[d][:, ts(h2, 512)]),
                            start=(d == 0), stop=False,
                        )
                for h2 in range(D // 512):
                    nc.tensor.matmul(
                        po[:, ts(h2, 512)],
                        _mm(ones_row[:]),
                        _mm(bo_row[:, ts(h2, 512)]),
                        start=False, stop=True,
                    )
                st = stage.tile([P, D], F32, tag="sto")
                nc.scalar.activation(st[:], po[:], AF.Copy)
                nc.sync.dma_start(out_d[ts(qt, P), :], st[:])


def make_in_maps(q, k, v, att_mask):
    """Build the 8 per-core input dicts from full inputs."""
    q = np.asarray(q, dtype=np.float32)
    k = np.asarray(k, dtype=np.float32)
    v = np.asarray(v, dtype=np.float32)
    att_mask = np.asarray(att_mask)
    in_maps = []
    kT_b = [np.ascontiguousarray(k[b].T) for b in range(B)]
    vT_b = [np.ascontiguousarray(v[b].T) for b in range(B)]
    for c in range(N_CORES):
        b, half = divmod(c, 2)
        qs = slice(half * SQ, (half + 1) * SQ)
        in_maps.append({
            "qT": np.ascontiguousarray(q[b, qs, :].T),
            "kT": kT_b[b],
            "vT": vT_b[b],
            "maskT": np.ascontiguousarray(
                att_mask[b, qs, :].T).astype(ml_dtypes.bfloat16),
        })
    return in_maps


_PROG = None


def _get_program():
    global _PROG
    if _PROG is None:
        _PROG = build_program()
    return _PROG


def kernel(q, k, v, att_mask, W_q, b_q, W_k, b_k, W_v, b_v, W_o, b_o,
           **_ignored):
    from concourse.bass_utils import run_bass_kernel_spmd

    nc = _get_program()
    weights = {
        "wq": np.ascontiguousarray(W_q, dtype=np.float32),
        "wk": np.ascontiguousarray(W_k, dtype=np.float32),
        "wv": np.ascontiguousarray(W_v, dtype=np.float32),
        "wo": np.ascontiguousarray(W_o, dtype=np.float32),
        "bq": np.ascontiguousarray(b_q, dtype=np.float32),
        "bk": np.ascontiguousarray(b_k, dtype=np.float32),
        "bv": np.ascontiguousarray(b_v, dtype=np.float32),
        "bo": np.ascontiguousarray(b_o, dtype=np.float32),
        "ones_row": np.ones((1, P), dtype=np.float32),
        "ones16": np.ones((P, H), dtype=np.float32),
    }
    in_maps = [dict(m, **weights) for m in make_in_maps(q, k, v, att_mask)]
    res = run_bass_kernel_spmd(nc, in_maps, core_ids=list(range(N_CORES)))
    out = np.empty((B, S, D), dtype=np.float32)
    for c in range(N_CORES):
        b, half = divmod(c, 2)
        out[b, half * SQ:(half + 1) * SQ, :] = res.results[c]["out"]
    return out

